# revision 76
# baseline (speedup 1.0000x reference)
# Trainium2 Bass kernel for nn_BidirRWKV6GaussianTimeMix.
# Sharding: 8 cores = (batch b, T-half). Each core computes 512 output tokens
# with a 128-token halo; the gaussian window (sigma~15) makes attention
# banded, so a banded attention over a 768-token extended window reproduces
# the reference.
# v2: bf16 data path (weights, x, mixes, k/r/v/g, attention factors) with the
# decay/cumsum/clip chain and groupnorm stats in fp32. All intermediates
# SBUF-resident. Attention masking is one wide mul per (head-pair, j-block)
# against a precomputed band mask in pat layout; the fwd/bwd diagonal split
# is absorbed by PSUM accumulation of the two y matmuls.
import numpy as np
import ml_dtypes

import concourse.bass as bass
import concourse.tile as tile
from concourse import mybir
from concourse.masks import make_identity

# ---------------------------------------------------------------------------
# Workaround: this walrus build rejects >1 sync-wait on a Drain instruction
# ("Too many sync wait commands"). Split the Tile tail drain into a chain of
# drains carrying one wait each.
_ORIG_DAB = tile.TileContext._drain_and_barrier
_WALRUS_FIXUPS = [True]

def _patched_dab(self, tick_clock, wait_clock):
    if not _WALRUS_FIXUPS[0]:
        return _ORIG_DAB(self, tick_clock, wait_clock)
    nc = self.nc
    import concourse.tile as _t
    drain_inst = nc.sync.drain()
    sc = _t.ScopedClock({None: tick_clock.global_clock})
    wait_clock.add_sem_waits(drain_inst.ins, sc)
    si = drain_inst.ins.sync_info
    waits = list(si.on_wait)
    if len(waits) > 1:
        drain_inst.ins.sync_info = type(si)(on_wait=waits[:1],
                                            on_update=list(si.on_update))
        for k in range(1, len(waits)):
            extra = nc.sync.drain()
            extra.ins.sync_info = type(si)(on_wait=waits[k:k + 1], on_update=[])
    nc.all_engine_barrier()
    assert self.sems is not None
    popped = nc._tile_sem_poison_stack.pop()
    assert popped is self._sem_poison
    nc.clear_and_free_semaphores(list(self.sems.allocated().values()))
    nc.all_engine_barrier()

tile.TileContext._drain_and_barrier = _patched_dab


_SPLIT_SEQ = [0]

def split_multi_waits(nc, max_waits=1):
    """Hoist excess sem-waits onto NOP carriers so no instruction carries
    more than max_waits waits (this walrus build's codegen limit)."""
    for f in nc.m.functions:
        for bb in f.blocks:
            il = list(bb.instructions)
            if not any(i.sync_info is not None and
                       len(i.sync_info.on_wait) > max_waits for i in il):
                continue
            new = []
            for ins in il:
                si = ins.sync_info
                if si is not None and len(si.on_wait) > max_waits:
                    w = list(si.on_wait)
                    excess, keep = w[:-max_waits], w[-max_waits:]
                    for k in range(0, len(excess), max_waits):
                        _SPLIT_SEQ[0] += 1
                        nop = mybir.InstNoOp(name=f"I-wsplit-{_SPLIT_SEQ[0]}",
                                             ins=[], outs=[])
                        nop.engine = ins.engine
                        nop.sync_info = mybir.SyncInfo(
                            on_wait=excess[k:k + max_waits], on_update=[])
                        new.append(nop)
                    ins.sync_info = mybir.SyncInfo(on_wait=keep,
                                                   on_update=list(si.on_update))
                new.append(ins)
            bb.instructions = new
# ---------------------------------------------------------------------------

B, T, D, H, K = 4, 1024, 1024, 16, 64
MID = 512
EPS = 1e-5 * 64.0
NB = D // 128          # 8 channel blocks
TEXT = 768             # uniform extended token window (6 blocks)
NT = TEXT // 128
CORE_LO = 128          # core tokens are ext cols [128, 640)
NCORE = 512
F32 = mybir.dt.float32
F32R = mybir.dt.float32r
BF = mybir.dt.bfloat16
ALU = mybir.AluOpType
AF = mybir.ActivationFunctionType

DEBUG_OUTS = ()


def _c0f(J):
    return min((max(J, 1) - 1) * 128, NCORE - 256)


def _c0b(J):
    return min((max(J - 1, 1) - 1) * 128, NCORE - 256)


def build_program(debug_outs=(), walrus_fixups=True):
    _WALRUS_FIXUPS[0] = walrus_fixups
    nc = bass.Bass()
    P = lambda n, s, dt=BF: nc.declare_dram_parameter(n, s, dt, isOutput=False)
    x_extT = P("x_extT", [D, TEXT + 2])   # channel-major (host-transposed)
    Wts = {n: P(n, [D, D]) for n in ["Wk", "Wv", "Wr", "Wg", "Wo"]}
    maa_w1 = P("maa_w1", [D, 160])
    maa_w2p = P("maa_w2p", [160, D])
    dw1_d = P("dw1", [D, 64])
    dw2_d = P("dw2", [64, D])
    vecs = P("vecs", [128, 72], F32)  # blocked: [:, bk*9:(bk+1)*9]
    bandmask = P("bandmask", [NT * 128, 1024])  # bf16, pat-layout per J
    rowmasks = P("rowmasks", [2, TEXT], F32)    # mmi, mme
    validb_d = P("validb", [1, TEXT])           # bf16 valid mask
    y_out = nc.declare_dram_parameter("y_out", [NCORE, D], BF, isOutput=True)

    dbg = {}
    def dbg_ap(name, shape, dt=F32):
        if name in debug_outs:
            dbg[name] = nc.declare_dram_parameter("dbg_" + name, shape, dt,
                                                  isOutput=True)
            return dbg[name]
        return None

    import contextlib
    lp = nc.allow_low_precision(reason="bf16 data path (rel-err budget 2e-2)")
    lp.__enter__()
    with tile.TileContext(nc) as tc, contextlib.ExitStack() as ctx:
        consts = ctx.enter_context(tc.tile_pool(name="consts", bufs=1))
        vecs_all = consts.tile([128, 72], F32, name="vecs_all")
        nc.sync.dma_start(out=vecs_all, in_=vecs[:, :])
        vecsT = [vecs_all[:, bk * 9:(bk + 1) * 9] for bk in range(NB)]
        w1sb = []
        for bk in range(NB):
            t_ = consts.tile([128, 160], BF, tag=f"w1_{bk}", name=f"w1_{bk}")
            w1sb.append(t_)
        w2A = consts.tile([64, D], BF, name="w2A")
        w2B = consts.tile([64, D], BF, name="w2B")
        w2C = consts.tile([32, D], BF, name="w2C")
        w2sb = [w2A[0:32, :], w2A[32:64, :], w2B[0:32, :], w2B[32:64, :], w2C]
        dw1sb = []
        for bk in range(NB):
            t_ = consts.tile([128, 64], BF, tag=f"dw1_{bk}", name=f"dw1_{bk}")
            dw1sb.append(t_)
        dw2sb = consts.tile([64, D], BF)

        def load_consts():
            # issued after the startup-critical x loads
            for bk in range(NB):
                nc.sync.dma_start(out=w1sb[bk],
                                  in_=maa_w1[bk * 128:(bk + 1) * 128, :])
            nc.sync.dma_start(out=w2A, in_=maa_w2p[0:64, :])
            nc.sync.dma_start(out=w2B, in_=maa_w2p[64:128, :])
            nc.sync.dma_start(out=w2C, in_=maa_w2p[128:160, :])
            for bk in range(NB):
                nc.sync.dma_start(out=dw1sb[bk],
                                  in_=dw1_d[bk * 128:(bk + 1) * 128, :])
            nc.sync.dma_start(out=dw2sb, in_=dw2_d[:, :])
        valid_b = consts.tile([128, TEXT], BF)

        epsc128 = consts.tile([128, 1], F32)
        nc.vector.memset(epsc128, EPS)
        c120 = consts.tile([128, 1], F32)
        nc.vector.memset(c120, 120.0)
        cm60 = consts.tile([128, 1], F32)
        nc.vector.memset(cm60, -60.0)
        cp60 = consts.tile([128, 1], F32)
        nc.vector.memset(cp60, 60.0)
        # Mb: block-diag [128,128], 1/64 within each head's 64x64 block;
        # Mb @ yT2 broadcasts per-head channel means onto the pair partitions.
        Mbf = consts.tile([128, 128], F32)
        nc.vector.memset(Mbf, 0.0)
        nc.vector.memset(Mbf[0:64, 0:64], 1.0 / 64.0)
        nc.vector.memset(Mbf[64:128, 64:128], 1.0 / 64.0)
        Mb = consts.tile([128, 128], F32)
        nc.vector.tensor_copy(out=Mb.bitcast(F32R), in_=Mbf)

        # persistent across phases (bf16, SBUF-resident)
        big = ctx.enter_context(tc.tile_pool(name="big", bufs=1))
        rT = [big.tile([128, NCORE], BF, tag=f"rT{i}", name=f"rT{i}") for i in range(NB)]
        kT_all = big.tile([128, NB, TEXT], BF, name="kT_all")
        gT = [big.tile([128, NCORE], BF, tag=f"gT{i}", name=f"gT{i}") for i in range(NB)]
        vS = big.tile([128, NT, D], BF, name="vS")
        zT = [big.tile([128, NCORE], BF, tag=f"zT{i}", name=f"zT{i}") for i in range(NB)]

        wep = ctx.enter_context(tc.tile_pool(name="wep", bufs=1))
        wexpT = [wep.tile([128, TEXT], F32, tag=f"we{i}", name=f"we{i}") for i in range(NB)]

        # cumsum/anchor tiles allocated up-front so the scans can be emitted
        # mid-phase-A (overlapping the projection matmuls)
        bigB = ctx.enter_context(tc.tile_pool(name="bigB", bufs=1))
        csT = [bigB.tile([128, TEXT], F32, tag=f"cs{i}", name=f"cs{i}") for i in range(NB)]
        Cf = [bigB.tile([128, 1], F32, tag=f"Cf{i}", name=f"Cf{i}") for i in range(NB)]
        Cb = [bigB.tile([128, 1], F32, tag=f"Cb{i}", name=f"Cb{i}") for i in range(NB)]
        cf60 = [bigB.tile([128, 1], F32, tag=f"cf6{i}", name=f"cf6{i}") for i in range(NB)]
        cb60 = [bigB.tile([128, 1], F32, tag=f"cb6{i}", name=f"cb6{i}") for i in range(NB)]
        cfp60 = [bigB.tile([128, 1], F32, tag=f"cfp{i}", name=f"cfp{i}") for i in range(NB)]
        cbp60 = [bigB.tile([128, 1], F32, tag=f"cbp{i}", name=f"cbp{i}") for i in range(NB)]

        # phase-B attention-factor pool, allocated OUTSIDE the phase-A region
        # so the per-head factor chain (SBUF-only DVE/Pool/Act work) can
        # schedule concurrently with the tail of phase A
        sfp = ctx.enter_context(tc.tile_pool(name="sfp", bufs=1))

        # ================= PHASE A ======================================
        with contextlib.ExitStack() as actx:
            pha = actx.enter_context(tc.tile_pool(name="pha", bufs=1))
            xT_all = pha.tile([128, NB, TEXT + 2], BF, name="xT_all")
            dxT_all = pha.tile([128, NB, TEXT], BF, name="dxT_all")
            xT = [xT_all[:, bk, :] for bk in range(NB)]
            dxT = [dxT_all[:, bk, :] for bk in range(NB)]
            xxx_A = pha.tile([64, TEXT], BF, name="xxx_A")
            xxx_B = pha.tile([64, TEXT], BF, name="xxx_B")
            xxx_C = pha.tile([32, TEXT], BF, name="xxx_C")

            # -- x arrives channel-major (host-transposed); derive dxprev
            # per channel block so the mix pipeline starts as soon as each
            # block's DMA lands --
            for bk in range(NB):
                nc.sync.dma_start(out=xT_all[:, bk, :],
                                  in_=x_extT[bk * 128:(bk + 1) * 128, :])


            mixp = actx.enter_context(tc.tile_pool(name="mixp", bufs=9))
            mmxp = actx.enter_context(tc.tile_pool(name="mmxp", bufs=3))
            # one shared phase-A psum pool: tag "mps" [64,384] x2 (3KB) +
            # tag "big" [128,2,512] x3 (12KB) = 15KB <= 16KB
            aps = actx.enter_context(tc.tile_pool(name="aps", bufs=1, space="PSUM"))
            wpool = actx.enter_context(tc.tile_pool(name="wpool", bufs=3))
            # anchor-mask pool: freed after the cumsum/anchor block (must be
            # top of the pool stack when released)
            anch = contextlib.ExitStack()
            anchp = anch.enter_context(tc.tile_pool(name="anchp", bufs=1))
            mmi_b = anchp.tile([128, TEXT], F32, name="mmi_b")
            mme_b = anchp.tile([128, TEXT], F32, name="mme_b")

            # deferred const loads (after the startup-critical x loads)
            load_consts()
            nc.sync.dma_start(out=mmi_b, in_=rowmasks[0:1, :].to_broadcast((128, TEXT)))
            nc.sync.dma_start(out=mme_b, in_=rowmasks[1:2, :].to_broadcast((128, TEXT)))
            nc.sync.dma_start(out=valid_b, in_=validb_d[0:1, :].to_broadcast((128, TEXT)))

            # -- dxprev + xxx = tanh(mix_x @ w1), pipelined per block --
            mixx = []
            for bk in range(NB):
                nc.vector.tensor_add(out=dxT_all[:, bk, :],
                                     in0=xT_all[:, bk, 0:TEXT],
                                     in1=xT_all[:, bk, 2:TEXT + 2])
                nc.vector.scalar_tensor_tensor(out=dxT_all[:, bk, :],
                                               in0=dxT_all[:, bk, :],
                                               scalar=0.5,
                                               in1=xT_all[:, bk, 1:TEXT + 1],
                                               op0=ALU.mult,
                                               op1=ALU.subtract)
                mx = mixp.tile([128, TEXT], BF, tag="mixs", name="mixs")
                nc.vector.scalar_tensor_tensor(out=mx, in0=dxT[bk],
                                               scalar=vecsT[bk][:, 0:1],
                                               in1=xT[bk][:, 1:1 + TEXT],
                                               op0=ALU.mult, op1=ALU.add)
                mixx.append(mx)
            for half in range(2):
                tsl = slice(half * 384, (half + 1) * 384)
                for dst, csl in ((xxx_A, slice(0, 64)), (xxx_B, slice(64, 128)),
                                 (xxx_C, slice(128, 160))):
                    psx_ = aps.tile([dst.shape[0], 384], F32, tag="mps",
                                    name="mps", bufs=2)
                    for bk in range(NB):
                        nc.tensor.matmul(psx_, w1sb[bk][:, csl], mixx[bk][:, tsl],
                                         start=(bk == 0), stop=(bk == NB - 1))
                    nc.scalar.activation(out=dst[:, tsl], in_=psx_, func=AF.Tanh)

            # -- five mix stages + projections --
            def make_mix(f):
                mixs = []
                xxf = (xxx_A[0:32], xxx_A[32:64], xxx_B[0:32], xxx_B[32:64],
                       xxx_C)[f]
                for bk in range(NB):
                    mx = mixp.tile([128, TEXT], BF, tag="mixs", name="mixs")
                    psm = aps.tile([128, 2, 512], F32, tag="big", name="big",
                                   bufs=3)
                    for half in range(2):
                        nc.tensor.matmul(psm[:, half, 0:384],
                                         w2sb[f][:, bk * 128:(bk + 1) * 128],
                                         xxf[:, half * 384:(half + 1) * 384],
                                         start=True, stop=True)
                    # one wide drain (+maa bias) -> bf16, then *dx, +x (DVE)
                    mmx = mmxp.tile([128, TEXT], BF, tag="mmx", name="mmx")
                    nc.scalar.activation(out=mmx, in_=psm[:, :, 0:384],
                                         func=AF.Identity,
                                         bias=vecsT[bk][:, 1 + f:2 + f])
                    nc.vector.tensor_mul(out=mx, in0=mmx, in1=dxT[bk])
                    eng = nc.gpsimd if bk % 2 else nc.vector
                    eng.tensor_add(out=mx, in0=mx, in1=xT[bk][:, 1:1 + TEXT])
                    mixs.append(mx)
                return mixs

            # channel-major projection helper; psum per do is a 2-bank
            # [128,2,512] tile (TEXT: two 384-chunks, NCORE: one 512-chunk)
            def proj_cm(Wd, mixs, tcols, post):
                for grp in ((0, 1), (2, 3), (4, 5), (6, 7)):
                    wt = {}
                    for bk in range(NB):
                        wt[bk] = wpool.tile([128, len(grp) * 128], BF, tag="wt", name="wt")
                        nc.sync.dma_start(
                            out=wt[bk],
                            in_=Wd[bk * 128:(bk + 1) * 128,
                                   grp[0] * 128:(grp[0] + len(grp)) * 128])
                    pss = {}
                    for gi in range(len(grp)):
                        pss[gi] = aps.tile([128, 2, 512], F32, tag="big",
                                           name="big", bufs=3)
                    for bk in range(NB):
                        for gi, do in enumerate(grp):
                            if tcols == TEXT:
                                for ci in range(2):
                                    nc.tensor.matmul(
                                        pss[gi][:, ci, 0:384],
                                        wt[bk][:, gi * 128:(gi + 1) * 128],
                                        mixs[bk][:, ci * 384:(ci + 1) * 384],
                                        start=(bk == 0), stop=(bk == NB - 1))
                            else:
                                nc.tensor.matmul(
                                    pss[gi][:, 0, :],
                                    wt[bk][:, gi * 128:(gi + 1) * 128],
                                    mixs[bk][:, CORE_LO:CORE_LO + 512],
                                    start=(bk == 0), stop=(bk == NB - 1))
                    for gi, do in enumerate(grp):
                        post(do, pss[gi])

            # f=0: w -> wexpT (fp32 chain)
            mixs = make_mix(0)
            h1 = mixp.tile([64, TEXT], BF, tag="h1", name="h1", bufs=1)
            for half in range(2):
                tsl = slice(half * 384, (half + 1) * 384)
                ph = aps.tile([64, 384], F32, tag="mps", name="mps", bufs=2)
                for bk in range(NB):
                    nc.tensor.matmul(ph, dw1sb[bk], mixs[bk][:, tsl],
                                     start=(bk == 0), stop=(bk == NB - 1))
                nc.scalar.activation(out=h1[:, tsl], in_=ph, func=AF.Tanh)
            for bk in range(NB):
                pw = aps.tile([128, 2, 512], F32, tag="big", name="big", bufs=3)
                for half in range(2):
                    nc.tensor.matmul(pw[:, half, 0:384],
                                     dw2sb[:, bk * 128:(bk + 1) * 128],
                                     h1[:, half * 384:(half + 1) * 384],
                                     start=True, stop=True)
                nc.scalar.activation(out=wexpT[bk], in_=pw[:, :, 0:384],
                                     func=AF.Exp, bias=vecsT[bk][:, 6:7])

            # cumsums + anchors emitted here: DVE work that overlaps the
            # projection matmuls below on PE. After the anchors, wexpT is
            # overwritten IN PLACE with csb = cs - wexp (Pool), which is what
            # phase B actually consumes; c*60 = 60 - C* feed the Act-side
            # relu-clip chain.
            with tc.tile_pool(name="scr", bufs=1) as scr:
                for bk in range(NB):
                    nc.vector.tensor_tensor_scan(out=csT[bk], data0=wexpT[bk],
                                                 data1=wexpT[bk], initial=0.0,
                                                 op0=ALU.add, op1=ALU.bypass)
                    s1 = scr.tile([128, TEXT], F32, tag="scr", name="scr")
                    nc.vector.scalar_tensor_tensor(out=s1, in0=wexpT[bk],
                                                   scalar=1.0,
                                                   in1=mmi_b, op0=ALU.mult,
                                                   op1=ALU.mult,
                                                   accum_out=Cf[bk])
                    s2 = scr.tile([128, TEXT], F32, tag="scr", name="scr")
                    nc.vector.scalar_tensor_tensor(out=s2, in0=wexpT[bk],
                                                   scalar=1.0,
                                                   in1=mme_b, op0=ALU.mult,
                                                   op1=ALU.mult,
                                                   accum_out=Cb[bk])
                    nc.gpsimd.tensor_sub(out=wexpT[bk], in0=csT[bk],
                                         in1=wexpT[bk])
                    nc.vector.tensor_scalar(out=cf60[bk], in0=Cf[bk],
                                            scalar1=-1.0, scalar2=60.0,
                                            op0=ALU.mult, op1=ALU.add)
                    nc.vector.tensor_scalar(out=cb60[bk], in0=Cb[bk],
                                            scalar1=-1.0, scalar2=60.0,
                                            op0=ALU.mult, op1=ALU.add)
                    nc.vector.tensor_scalar_add(out=cfp60[bk], in0=Cf[bk],
                                                scalar1=60.0)
                    nc.vector.tensor_scalar_add(out=cbp60[bk], in0=Cb[bk],
                                                scalar1=60.0)
            anch.close()
            csbT = wexpT  # renamed: holds cs - wexp from here on

            # phase-B per-head factor chain; emitted early (software-
            # pipelined) so Act/DVE/Pool fill gaps while PE runs projections
            chains = {}

            def emit_chain(hb, clip_on_act=False):
                # s = clip(x - C, -60, 60); the negated clip r2 = 60 - s can
                # run on Act (two Relus, affine folded into the Exp) or on
                # DVE (tensor_scalar), chosen by which engine has slack.
                uf = sfp.tile([128, TEXT], F32, tag="uf", name="uf")
                ub = sfp.tile([128, TEXT], F32, tag="ub", name="ub")
                if clip_on_act:
                    # u = relu(x + (60 - C)); r2 = relu(120 - u)
                    nc.scalar.activation(out=uf, in_=csT[hb], func=AF.Relu,
                                         bias=cf60[hb])
                    nc.scalar.activation(out=uf, in_=uf, func=AF.Relu,
                                         scale=-1.0, bias=c120)
                    nc.scalar.activation(out=ub, in_=csbT[hb], func=AF.Relu,
                                         bias=cb60[hb])
                    nc.scalar.activation(out=ub, in_=ub, func=AF.Relu,
                                         scale=-1.0, bias=c120)
                else:
                    # r2 = 60 - s = clip((C + 60) - x, 0, 120) on DVE
                    nc.vector.tensor_scalar(out=uf, in0=csT[hb],
                                            scalar1=cfp60[hb], scalar2=-1.0,
                                            op0=ALU.subtract, op1=ALU.mult)
                    nc.vector.tensor_scalar(out=uf, in0=uf,
                                            scalar1=120.0, scalar2=0.0,
                                            op0=ALU.min, op1=ALU.max)
                    nc.vector.tensor_scalar(out=ub, in0=csbT[hb],
                                            scalar1=cbp60[hb], scalar2=-1.0,
                                            op0=ALU.subtract, op1=ALU.mult)
                    nc.vector.tensor_scalar(out=ub, in0=ub,
                                            scalar1=120.0, scalar2=0.0,
                                            op0=ALU.min, op1=ALU.max)
                # exp factors (bf16); efm/ebp only needed on core columns
                efm = sfp.tile([128, NCORE], BF, tag="efm", name="efm")
                nc.scalar.activation(out=efm, in_=uf[:, CORE_LO:CORE_LO + NCORE],
                                     func=AF.Exp, bias=cm60)
                efp = sfp.tile([128, TEXT], BF, tag="efp", name="efp")
                nc.scalar.activation(out=efp, in_=uf, func=AF.Exp,
                                     scale=-1.0, bias=cp60)
                ebm = sfp.tile([128, TEXT], BF, tag="ebm", name="ebm")
                nc.scalar.activation(out=ebm, in_=ub, func=AF.Exp,
                                     bias=cm60)
                ebp = sfp.tile([128, NCORE], BF, tag="ebp", name="ebp")
                nc.scalar.activation(out=ebp, in_=ub[:, CORE_LO:CORE_LO + NCORE],
                                     func=AF.Exp, scale=-1.0, bias=cp60)
                Kf = sfp.tile([128, TEXT], BF, tag="Kf", name="Kf", bufs=3)
                nc.vector.tensor_mul(out=Kf, in0=kT_all[:, hb, :], in1=efp)
                Kb = sfp.tile([128, TEXT], BF, tag="Kb", name="Kb", bufs=3)
                nc.gpsimd.tensor_mul(out=Kb, in0=kT_all[:, hb, :], in1=ebm)
                Rf = sfp.tile([128, NCORE], BF, tag="Rf", name="Rf", bufs=3)
                nc.vector.tensor_mul(out=Rf, in0=rT[hb], in1=efm)
                Rb = sfp.tile([128, NCORE], BF, tag="Rb", name="Rb", bufs=3)
                nc.gpsimd.tensor_mul(out=Rb, in0=rT[hb], in1=ebp)
                chains[hb] = (Kf, Kb, Rf, Rb)

            # f=1: k (masked by valid)
            mixs = make_mix(1)
            def post_k(do, ps):
                nc.scalar.activation(out=kT_all[:, do, :], in_=ps[:, :, 0:384],
                                     func=AF.Copy)
            proj_cm(Wts["Wk"], mixs, TEXT, post_k)
            for do in range(NB):
                nc.vector.tensor_mul(out=kT_all[:, do, :],
                                     in0=kT_all[:, do, :], in1=valid_b)

            # f=3: r
            mixs = make_mix(3)
            def post_r(do, ps):
                nc.vector.tensor_copy(out=rT[do], in_=ps[:, 0, :])
            proj_cm(Wts["Wr"], mixs, NCORE, post_r)

            # f=2: v (token-major, SBUF-resident), in channel-half waves so
            # the low head-pairs' v is ready before the second wave
            mixs = make_mix(2)
            emit_chain(0)
            emit_chain(1)
            for half in range(2):
                pss = {}
                for tp in range(NT // 2):
                    pss[tp] = aps.tile([128, 2, 512], F32, tag="big",
                                       name="big", bufs=3)
                for bk in range(NB):
                    wvh = wpool.tile([128, 512], BF, tag="wtv", name="wtv")
                    nc.sync.dma_start(
                        out=wvh,
                        in_=Wts["Wv"][bk * 128:(bk + 1) * 128,
                                      half * 512:(half + 1) * 512])
                    for tp in range(NT // 2):
                        for t in range(2):
                            nc.tensor.matmul(
                                pss[tp][:, t, :],
                                mixs[bk][:, (tp * 2 + t) * 128:(tp * 2 + t + 1) * 128],
                                wvh, start=(bk == 0), stop=(bk == NB - 1))
                for tp in range(NT // 2):
                    nc.scalar.activation(
                        out=vS[:, tp * 2:tp * 2 + 2, half * 512:(half + 1) * 512],
                        in_=pss[tp], func=AF.Copy)

            # f=4: g (silu) -- last so the act-table switch happens once
            emit_chain(2)
            mixs = make_mix(4)
            def post_g(do, ps):
                nc.scalar.activation(out=gT[do], in_=ps[:, 0, :], func=AF.Silu)
            proj_cm(Wts["Wg"], mixs, NCORE, post_g)

        if "csT" in debug_outs:
            d = dbg_ap("csT", [NB * 128, TEXT])
            for bk in range(NB):
                nc.sync.dma_start(out=d[bk * 128:(bk + 1) * 128, :], in_=csT[bk])
        if "kT" in debug_outs:
            d = dbg_ap("kT", [NB * 128, TEXT], BF)
            for bk in range(NB):
                nc.sync.dma_start(out=d[bk * 128:(bk + 1) * 128, :],
                                  in_=kT_all[:, bk, :])
        if "rT" in debug_outs:
            d = dbg_ap("rT", [NB * 128, NCORE], BF)
            for bk in range(NB):
                nc.sync.dma_start(out=d[bk * 128:(bk + 1) * 128, :], in_=rT[bk])
        if "gT" in debug_outs:
            d = dbg_ap("gT", [NB * 128, NCORE], BF)
            for bk in range(NB):
                nc.sync.dma_start(out=d[bk * 128:(bk + 1) * 128, :], in_=gT[bk])
        if "vS" in debug_outs:
            d = dbg_ap("vS", [NT * 128, D], BF)
            for tb in range(NT):
                nc.sync.dma_start(out=d[tb * 128:(tb + 1) * 128, :],
                                  in_=vS[:, tb, :])
        if "wexpT" in debug_outs:
            d = dbg_ap("wexpT", [NB * 128, TEXT])
            for bk in range(NB):
                nc.sync.dma_start(out=d[bk * 128:(bk + 1) * 128, :], in_=wexpT[bk])

        # ================= PHASE B: attention per head-pair ==============
        with contextlib.ExitStack() as bctx:
            hp = bctx.enter_context(tc.tile_pool(name="hp", bufs=1))
            atp = bctx.enter_context(tc.tile_pool(name="atp", bufs=4))
            bmp = bctx.enter_context(tc.tile_pool(name="bmp", bufs=1))
            bandm = []
            for J in range(NT):
                t_ = bmp.tile([128, 2, 512], BF, tag=f"bm{J}", name=f"bm{J}")
                nc.sync.dma_start(out=t_, in_=bandmask[J * 128:(J + 1) * 128, :])
                bandm.append(t_)
            wpo = bctx.enter_context(tc.tile_pool(name="wpo", bufs=8))
            ostg = bctx.enter_context(tc.tile_pool(name="ostg", bufs=3))
            b_ps = contextlib.ExitStack()
            at_ps = b_ps.enter_context(tc.tile_pool(name="at_ps", bufs=2, space="PSUM"))
            y_ps = b_ps.enter_context(tc.tile_pool(name="y_ps", bufs=1, space="PSUM"))
            st_ps = b_ps.enter_context(tc.tile_pool(name="st_ps", bufs=1, space="PSUM"))

            # phase C's Wo loads issued now so the DMA is long done by then
            wts = []
            for bk in range(NB):
                wt = wpo.tile([128, D], BF, tag="wo", name="wo")
                nc.sync.dma_start(out=wt, in_=Wts["Wo"][bk * 128:(bk + 1) * 128, :])
                wts.append(wt)

            ydbg = dbg_ap("yT", [H * 64, NCORE]) if "yT" in debug_outs else None

            for hb in range(H // 2):
                Kf, Kb, Rf, Rb = chains.pop(hb)

                # banded attention, uniform loop; fwd/bwd y matmuls both
                # accumulate into py2 so the diagonal split needs no adds.
                py2 = y_ps.tile([128, 512], F32, tag="py2", name="py2")
                nc.vector.memset(py2, 0.0)
                for J in range(NT):
                    c0f, c0b = _c0f(J), _c0b(J)
                    fwd_live = J < NT - 1   # J=5 fwd is fully masked
                    bwd_live = J > 0        # J=0 bwd is fully masked
                    pat2 = at_ps.tile([128, 2, 512], F32, tag="pat", name="pat")
                    for h2 in range(2):
                        psl = slice(h2 * 64, h2 * 64 + 64)
                        if fwd_live:
                            nc.tensor.matmul(pat2[:, h2, 0:256],
                                             Kf[psl, J * 128:(J + 1) * 128],
                                             Rf[psl, c0f:c0f + 256],
                                             start=True, stop=True)
                        if bwd_live:
                            nc.tensor.matmul(pat2[:, h2, 256:512],
                                             Kb[psl, J * 128:(J + 1) * 128],
                                             Rb[psl, c0b:c0b + 256],
                                             start=True, stop=True)
                    at2 = atp.tile([128, 2, 512], BF, tag="at", name="at")
                    if fwd_live and bwd_live:
                        sl = (slice(None), slice(None), slice(0, 512))
                    elif fwd_live:
                        sl = (slice(None), slice(None), slice(0, 256))
                    else:
                        sl = (slice(None), slice(None), slice(256, 512))
                    if (hb + J) % 3 != 2:
                        nc.vector.tensor_mul(out=at2[sl], in0=pat2[sl],
                                             in1=bandm[J][sl])
                    else:
                        # route via Act copy so the mul runs at bf16 2x rate
                        pc = atp.tile([128, 2, 512], BF, tag="pc", name="pc")
                        nc.scalar.activation(out=pc[sl], in_=pat2[sl],
                                             func=AF.Copy)
                        nc.vector.tensor_mul(out=at2[sl], in0=pc[sl],
                                             in1=bandm[J][sl])
                    for h2 in range(2):
                        if fwd_live:
                            nc.tensor.matmul(
                                py2[h2 * 64:(h2 + 1) * 64, c0f:c0f + 256],
                                vS[:, J, hb * 128 + h2 * 64:hb * 128 + (h2 + 1) * 64],
                                at2[:, h2, 0:256],
                                start=False, stop=False,
                                skip_group_check=True,
                                tile_position=(0, h2 * 64))
                        if bwd_live:
                            nc.tensor.matmul(
                                py2[h2 * 64:(h2 + 1) * 64, c0b:c0b + 256],
                                vS[:, J, hb * 128 + h2 * 64:hb * 128 + (h2 + 1) * 64],
                                at2[:, h2, 256:512],
                                start=False, stop=(J == NT - 1),
                                skip_group_check=True,
                                tile_position=(0, h2 * 64))

                # software pipeline: emit the factor chain 3 head-pairs
                # ahead (clips on Act there: DVE is the busy engine in B)
                if hb + 3 < H // 2:
                    emit_chain(hb + 3, clip_on_act=True)

                # groupnorm + gate; rsqrt via Ln+Exp (stays on the
                # natural_log_exp act table -- no table thrash)
                yT2 = hp.tile([128, 512], F32, tag="yT2", name="yT2")
                nc.scalar.activation(out=yT2.bitcast(F32R), in_=py2, func=AF.Copy)
                ysq = hp.tile([128, 512], F32, tag="ysq", name="ysq")
                nc.scalar.activation(out=ysq.bitcast(F32R), in_=py2, func=AF.Square)
                pmb = st_ps.tile([128, 512], F32, tag="pmb", name="pmb")
                nc.tensor.matmul(pmb, Mb.bitcast(F32R), yT2.bitcast(F32R),
                                 start=True, stop=True)
                pms2 = st_ps.tile([128, 512], F32, tag="pms2", name="pms2")
                nc.tensor.matmul(pms2, Mb.bitcast(F32R), ysq.bitcast(F32R),
                                 start=True, stop=True)
                msq = hp.tile([128, 512], F32, tag="msq", name="msq")
                nc.scalar.activation(out=msq, in_=pmb, func=AF.Square)
                var2 = hp.tile([128, 512], F32, tag="var2", name="var2")
                nc.vector.tensor_sub(out=var2, in0=pms2, in1=msq)
                lnv = hp.tile([128, 512], F32, tag="lnv", name="lnv")
                nc.scalar.activation(out=lnv, in_=var2, func=AF.Ln,
                                     bias=epsc128)
                rstd2 = hp.tile([128, 512], F32, tag="rstd2", name="rstd2")
                nc.scalar.activation(out=rstd2, in_=lnv, func=AF.Exp,
                                     scale=-0.5)
                zh = hp.tile([128, NCORE], F32, tag="zh", name="zh")
                nc.vector.tensor_sub(out=zh, in0=yT2, in1=pmb)
                nc.gpsimd.tensor_mul(out=zh, in0=zh, in1=rstd2)
                zh2 = hp.tile([128, NCORE], BF, tag="zh2", name="zh2", bufs=2)
                nc.vector.tensor_scalar(out=zh2, in0=zh,
                                        scalar1=vecsT[hb][:, 7:8],
                                        scalar2=vecsT[hb][:, 8:9],
                                        op0=ALU.mult, op1=ALU.add)
                nc.vector.tensor_mul(out=zT[hb], in0=zh2, in1=gT[hb])

                if ydbg is not None:
                    nc.sync.dma_start(out=ydbg[hb * 128:(hb + 1) * 128, :],
                                      in_=yT2)

            # ============= PHASE C: out = z @ Wo =========================
            b_ps.close()
            o_ps = bctx.enter_context(tc.tile_pool(name="o_ps", bufs=4, space="PSUM"))
            # two token-pair waves: the drains of wave 0 overlap wave 1's MMs
            for wave in range(2):
                pss = {}
                for tb2 in range(2):
                    for half in range(2):
                        pss[(tb2, half)] = o_ps.tile([128, 512], F32, tag="po",
                                                     name="po")
                for bk in range(NB):
                    for tb2 in range(2):
                        tb = wave * 2 + tb2
                        for half in range(2):
                            nc.tensor.matmul(pss[(tb2, half)],
                                             zT[bk][:, tb * 128:(tb + 1) * 128],
                                             wts[bk][:, half * 512:(half + 1) * 512],
                                             start=(bk == 0), stop=(bk == NB - 1))
                for tb2 in range(2):
                    tb = wave * 2 + tb2
                    ot = ostg.tile([128, D], BF, tag="ot", name="ot")
                    for half in range(2):
                        if half == 0:
                            nc.vector.tensor_copy(
                                out=ot[:, half * 512:(half + 1) * 512],
                                in_=pss[(tb2, half)])
                        else:
                            nc.scalar.activation(
                                out=ot[:, half * 512:(half + 1) * 512],
                                in_=pss[(tb2, half)], func=AF.Copy)
                    nc.sync.dma_start(out=y_out[tb * 128:(tb + 1) * 128, :], in_=ot)

    if walrus_fixups:
        split_multi_waits(nc)
    _WALRUS_FIXUPS[0] = True
    return nc, dbg


_PROGRAM_CACHE = {}


def _get_program(debug_outs=(), walrus_fixups=True):
    key = (tuple(debug_outs), walrus_fixups)
    if key not in _PROGRAM_CACHE:
        _PROGRAM_CACHE[key] = build_program(debug_outs, walrus_fixups)
    return _PROGRAM_CACHE[key]


BF_NP = ml_dtypes.bfloat16


def make_in_maps(inputs):
    x = np.asarray(inputs["x"], np.float32)
    softplus = lambda v: np.log1p(np.exp(v.astype(np.float64)))
    mu = softplus(np.asarray(inputs["gauss_mu_raw"]))
    sigma = softplus(np.asarray(inputs["gauss_sigma_raw"]))
    assert np.allclose(mu, mu[0]) and np.allclose(sigma, sigma[0]), \
        "per-head gaussian masks not supported by this kernel build"
    gauss = lambda d: np.exp(-0.5 * ((np.abs(d) - mu[0]) / sigma[0]) ** 2)
    # band mask in pat layout: per J, [128 j_rel, 2 h2, 512] with
    # [0:256]=fwd window (i>=j), [256:512]=bwd window (i<j); both heads equal
    bandmask = np.zeros((NT, 128, 2, 512), np.float64)
    jj = np.arange(128)[:, None]
    ii = np.arange(256)[None, :]
    for J in range(NT):
        j_ext = J * 128 + jj
        i_f = CORE_LO + _c0f(J) + ii
        i_b = CORE_LO + _c0b(J) + ii
        fw = gauss(i_f - j_ext) * (i_f >= j_ext)
        bw = gauss(i_b - j_ext) * (i_b < j_ext)
        if J == NT - 1:
            fw = fw * 0.0
        if J == 0:
            bw = bw * 0.0
        for h2 in range(2):
            bandmask[J, :, h2, 0:256] = fw
            bandmask[J, :, h2, 256:512] = bw
    bandmask = bandmask.reshape(NT * 128, 1024).astype(BF_NP)
    vecs = np.stack([np.asarray(inputs[n], np.float32).reshape(-1) for n in
                     ["time_maa_x", "time_maa_w", "time_maa_k", "time_maa_v",
                      "time_maa_r", "time_maa_g", "time_decay", "ln_w", "ln_b"]],
                    axis=1)
    vecs = vecs.reshape(8, 128, 9).transpose(1, 0, 2).reshape(128, 72)
    shared = {
        "Wk": np.asarray(inputs["W_k"], np.float32).astype(BF_NP),
        "Wv": np.asarray(inputs["W_v"], np.float32).astype(BF_NP),
        "Wr": np.asarray(inputs["W_r"], np.float32).astype(BF_NP),
        "Wg": np.asarray(inputs["W_g"], np.float32).astype(BF_NP),
        "Wo": np.asarray(inputs["W_o"], np.float32).astype(BF_NP),
        "maa_w1": np.asarray(inputs["time_maa_w1"], np.float32).astype(BF_NP),
        "maa_w2p": np.asarray(inputs["time_maa_w2"],
                              np.float32).reshape(160, D).astype(BF_NP),
        "dw1": np.asarray(inputs["time_decay_w1"], np.float32).astype(BF_NP),
        "dw2": np.asarray(inputs["time_decay_w2"], np.float32).astype(BF_NP),
        "vecs": np.ascontiguousarray(vecs),
        "bandmask": np.ascontiguousarray(bandmask),
    }
    in_maps = []
    for c in range(8):
        b, half = c // 2, c % 2
        t0 = half * 512
        e0 = t0 - 128
        xe = np.zeros((TEXT + 2, D), np.float32)
        glo, ghi = max(0, e0 - 1), min(T, e0 + TEXT + 1)
        xe[glo - (e0 - 1):ghi - (e0 - 1)] = x[b, glo:ghi]
        mid_l = MID - e0
        tt = np.arange(TEXT)
        rowmasks = np.stack([
            (tt <= mid_l).astype(np.float32),
            (tt <= mid_l - 1).astype(np.float32)], axis=0)
        validb = ((tt + e0 >= 0) & (tt + e0 < T)).astype(np.float32)
        m = dict(shared)
        m["x_extT"] = np.ascontiguousarray(xe.astype(BF_NP).T)
        m["rowmasks"] = np.ascontiguousarray(rowmasks)
        m["validb"] = np.ascontiguousarray(validb[None, :]).astype(BF_NP)
        in_maps.append(m)
    return in_maps


def run_cores(inputs, debug_outs=(), trace=False):
    from concourse.bass_utils import run_bass_kernel_spmd
    in_maps = make_in_maps(inputs)
    nc, dbg = _get_program(debug_outs)
    res = run_bass_kernel_spmd(nc, in_maps, list(range(8)), trace=trace)
    return res


def kernel(**inputs):
    res = run_cores(inputs)
    out = np.zeros((B, T, D), np.float32)
    for c in range(8):
        b, half = c // 2, c % 2
        out[b, half * 512:(half + 1) * 512] = \
            np.asarray(res.results[c]["y_out"], np.float32)
    return out
